# revision 1
# baseline (speedup 1.0000x reference)
"""Trainium2 Bass kernel for a pre-LN transformer block (B=4, T=2048, C=1024,
H=16 heads, MLP 4x), data-parallel over 8 NeuronCores.

Sharding: core c <- (batch b = c//2, parity = c%2); each core handles query
rows x[b, parity::2] plus LayerNorm1 + K/V projections over the full T=2048
rows of its batch (duplicated across the parity pair; no collectives).

v2: all projection/MLP matmuls run in fp8 (e4m3) DoubleRow mode (2 contraction
k-tiles per pass = 2x tensor throughput), attention scores in fp8, exp output
fp8, attention-V in fp8 DoubleRow. Weights are host-prescaled by 32 (fp8
dynamic range) and the 1/32 is folded into the epilogues. Activations bf16
where precision matters (x, LN stats, residuals). K/V/x2 stay in SBUF (no
DRAM round trips). Causal structure: fully-masked key chunks skipped at
compile time; diagonal-band matmul/exp restricted to valid column sub-ranges
with a single shared [128,64] wedge mask.
"""

import numpy as np
from contextlib import ExitStack

import concourse.bass as bass
import concourse.bacc as bacc
import concourse.tile as tile
from concourse import mybir
from concourse.bass_utils import run_bass_kernel_spmd

F32 = mybir.dt.float32
F32R = mybir.dt.float32r
BF16 = mybir.dt.bfloat16
FP8 = mybir.dt.float8e4
AF = mybir.ActivationFunctionType
ALU = mybir.AluOpType
DR = mybir.MatmulPerfMode.DoubleRow

C = 1024          # embedding dim
T = 2048          # kv sequence length per core
TQ = 1024         # query rows per core
H = 16            # heads
D = 64            # head dim
HID = 4096        # mlp hidden
P = 128           # partitions
CC = C // P       # 8 feature chunks
NKT = T // P      # 16 key chunks
NQT = TQ // 512   # 2 query tiles of 512
NPAIR = H // 2    # 8 head pairs (= feature chunks)
NHC = HID // P    # 32 hidden chunks
EPS = 1e-5
WS = 32.0         # host-side fp8 weight prescale
SINV = 1.0 / WS
MASK_VAL = -8.0e9
EXP_SCALE = 0.125 / (WS * WS)   # q,k both carry x32

N_CORES = 8
VW = 80           # padded per-head vg width (64 v dims + aug@64 + pad); %16==0


def _ap_stride2(t2, off):
    """[P, n] view of a 2-D AP selecting every 2nd free element from off."""
    return bass.AP(tensor=t2.tensor, offset=t2.offset + off,
                   ap=[list(t2.ap[0]), [2, TQ]])


def _bcast2(dram_ap, parts, rows, n):
    """[parts, rows, n] AP broadcasting a [rows, n] dram region across parts."""
    return bass.AP(tensor=dram_ap.tensor, offset=dram_ap.offset,
                   ap=[[0, parts], [n, rows], [1, n]])


def build_program(phases=None):
    import os
    if phases is None:
        phases = int(os.environ.get("BASS_BUILD_PHASES", "5"))
    iters = int(os.environ.get("BASS_BENCH_ITERS", "1"))
    nc = bacc.Bacc("TRN2", target_bir_lowering=False, debug=False)

    xkvT = nc.declare_dram_parameter("xkvT", [C, T], BF16, False)
    xqF = nc.declare_dram_parameter("xqF", [C, TQ], F32, False)
    wedge = nc.declare_dram_parameter("wedge", [P, D], F32, False)
    Wq8 = nc.declare_dram_parameter("Wq8", [512, 2 * C], FP8, False)
    Wk8 = nc.declare_dram_parameter("Wk8", [512, 2 * C], FP8, False)
    Wv8 = nc.declare_dram_parameter("Wv8", [512, 2 * C], FP8, False)
    WoB = nc.declare_dram_parameter("WoB", [C, C], BF16, False)
    W1B = nc.declare_dram_parameter("W1B", [C, HID], BF16, False)
    W28 = nc.declare_dram_parameter("W28", [2048, 2 * C], FP8, False)
    bq32 = nc.declare_dram_parameter("bq32", [C], F32, False)
    bk32 = nc.declare_dram_parameter("bk32", [C], F32, False)
    bv32 = nc.declare_dram_parameter("bv32", [C], F32, False)
    bo = nc.declare_dram_parameter("bo", [C], F32, False)
    b1 = nc.declare_dram_parameter("b1", [HID], F32, False)
    b2 = nc.declare_dram_parameter("b2", [C], F32, False)
    ln1_g = nc.declare_dram_parameter("ln1_g", [C], F32, False)
    ln1_b = nc.declare_dram_parameter("ln1_b", [C], F32, False)
    ln2_g = nc.declare_dram_parameter("ln2_g", [C], F32, False)
    ln2_b = nc.declare_dram_parameter("ln2_b", [C], F32, False)
    outT = nc.declare_dram_parameter("outT", [C, TQ], F32, True)

    ln_scratch = nc.dram_tensor("ln_scratch", [2, T], F32)
    ln2_scratch = nc.dram_tensor("ln2_scratch", [2, TQ], F32)
    lnq_scratch = nc.dram_tensor("lnq_scratch", [2, TQ], F32)

    with tile.TileContext(nc) as tc, ExitStack() as ctx:
        ctx.enter_context(nc.allow_low_precision(
            reason="fp8/bf16 kernel; tolerance 2e-2"))
        consts = ctx.enter_context(tc.tile_pool(name="consts", bufs=1))

        def load_cols(v, n, nm):
            t = consts.tile([P, n], F32, tag=nm, name=nm)
            nc.sync.dma_start(out=t, in_=v.rearrange("(k p) -> p k", p=P))
            return t

        bq_c = load_cols(bq32, CC, "bq_c")
        bk_c = load_cols(bk32, CC, "bk_c")
        bo_c = load_cols(bo, CC, "bo_c")
        b2_c = load_cols(b2, CC, "b2_c")
        b1_c = load_cols(b1, NHC, "b1_c")
        l1g_c = load_cols(ln1_g, CC, "l1g_c")
        l1b_c = load_cols(ln1_b, CC, "l1b_c")
        l2g_c = load_cols(ln2_g, CC, "l2g_c")
        l2b_c = load_cols(ln2_b, CC, "l2b_c")
        bv_b = consts.tile([P, C], F32, tag="bv_b", name="bv_b")
        nc.sync.dma_start(out=bv_b, in_=_bcast2(bv32[:], P, 1, C))
        wedge_t = consts.tile([P, D], F32, tag="wedge", name="wedge")
        nc.sync.dma_start(out=wedge_t, in_=wedge[:, :])
        ones_bf = consts.tile([P, 1], BF16, tag="ones_bf", name="ones_bf")
        nc.scalar.activation(ones_bf, bv_b[:, 0:1], AF.Copy, bias=1.0, scale=0.0)
        ones_r = consts.tile([P, 1], F32, tag="ones_r", name="ones_r")
        nc.scalar.activation(ones_r.bitcast(F32R), bv_b[:, 0:1], AF.Copy,
                             bias=1.0, scale=0.0)
        ones_all = consts.tile([P, D], BF16, tag="ones_all", name="ones_all")
        nc.scalar.activation(ones_all, bv_b[:, 0:D], AF.Copy, bias=1.0, scale=0.0)
        eps_t = consts.tile([1, 1], F32, tag="eps", name="eps")
        nc.vector.memset(eps_t, EPS)
        one_c = consts.tile([P, 1], F32, tag="one_c", name="one_c")
        nc.vector.memset(one_c, 1.0)
        sinv_c = consts.tile([P, 1], F32, tag="sinv_c", name="sinv_c")
        nc.vector.memset(sinv_c, SINV)
        vgB = ctx.enter_context(tc.tile_pool(name="vgB", bufs=1))
        vg8 = [vgB.tile([P, NKT, 2 * VW], FP8, tag=f"vg{pr}",
                        name=f"vg{pr}") for pr in range(NPAIR)]
        for pr in range(NPAIR):
            nc.vector.memset(vg8[pr], 0.0)
            nc.vector.memset(
                bass.AP(tensor=vg8[pr].tensor, offset=vg8[pr].offset + D,
                        ap=[list(vg8[pr].ap[0]), [2 * VW, NKT], [VW, 2]]), WS)

        def _ln_stats(xt3, t_len, scratch, pool, nm):
            """Column-wise LN stats of bf16 [P, CC, t_len] tile -> broadcast
            tile ab [P, 2, t_len] fp32 (row0: rstd, row1: mean*rstd)."""
            ntt = t_len // 512
            with tc.tile_pool(name=f"{nm}_sq", bufs=2) as sqp, \
                 tc.tile_pool(name=f"{nm}_ps", bufs=1, space="PSUM") as psp:
                sum_ps = [psp.tile([1, 512], F32, tag=f"{nm}_su{t}",
                                   name=f"{nm}_su{t}") for t in range(ntt)]
                sq_ps = [psp.tile([1, 512], F32, tag=f"{nm}_sq{t}",
                                  name=f"{nm}_sq{t}") for t in range(ntt)]
                is_f32 = xt3.dtype != BF16
                ones_t = ones_r.bitcast(F32R) if is_f32 else ones_bf
                sdt = F32 if is_f32 else BF16
                for ci in range(CC):
                    xt = xt3[:, ci, :]
                    if is_f32:
                        xt = xt.bitcast(F32R)
                    sq = sqp.tile([P, t_len], sdt, tag="sq", name=f"{nm}_sqt")
                    if is_f32:
                        nc.scalar.activation(sq.bitcast(F32R), xt3[:, ci, :],
                                             AF.Square)
                        sq = sq.bitcast(F32R)
                    else:
                        nc.scalar.activation(sq, xt, AF.Square)
                    for t in range(ntt):
                        sl = slice(t * 512, (t + 1) * 512)
                        nc.tensor.matmul(sum_ps[t], ones_t, xt[:, sl],
                                         start=(ci == 0), stop=(ci == CC - 1))
                        nc.tensor.matmul(sq_ps[t], ones_t, sq[:, sl],
                                         start=(ci == 0), stop=(ci == CC - 1))
                mean = pool.tile([1, t_len], F32, tag=f"{nm}_mean",
                                 name=f"{nm}_mean")
                exsq = pool.tile([1, t_len], F32, tag=f"{nm}_exsq",
                                 name=f"{nm}_exsq")
                for t in range(ntt):
                    sl = slice(t * 512, (t + 1) * 512)
                    nc.scalar.activation(mean[:, sl], sum_ps[t], AF.Copy,
                                         scale=1.0 / C)
                    nc.scalar.activation(exsq[:, sl], sq_ps[t], AF.Copy,
                                         scale=1.0 / C)
            a_row = pool.tile([1, t_len], F32, tag=f"{nm}_a", name=f"{nm}_a")
            nc.vector.tensor_mul(a_row, mean, mean)
            nc.vector.tensor_sub(exsq, exsq, a_row)             # var
            nc.scalar.activation(exsq, exsq, AF.Sqrt, bias=eps_t)
            nc.vector.reciprocal(a_row, exsq)                   # rstd
            nc.vector.tensor_mul(exsq, mean, a_row)             # mean*rstd
            nc.gpsimd.dma_start(out=scratch[0:1, :], in_=a_row)
            nc.gpsimd.dma_start(out=scratch[1:2, :], in_=exsq)
            ab = pool.tile([P, 2, t_len], F32, tag=f"{nm}_ab", name=f"{nm}_ab")
            nc.sync.dma_start(out=ab, in_=_bcast2(scratch[:, :], P, 2, t_len))
            return ab

        def _emit_body():
            with tc.tile_pool(name="x2B", bufs=1) as x2B:
                with tc.tile_pool(name="hB", bufs=1) as hB:
                    hT = hB.tile([P, CC, T], FP8, tag="hT", name="hT")
                    hq = hB.tile([P, CC, TQ], FP8, tag="hq", name="hq")
                    xqf = hB.tile([P, CC, TQ], F32, tag="xqf", name="xqf")
                    for ci in range(CC):
                        nc.sync.dma_start(
                            out=xqf[:, ci, :].bitcast(F32R),
                            in_=xqF[ci * P:(ci + 1) * P, :].bitcast(F32R))

                    # LN1 on query rows (parity baked into xq data)
                    with tc.tile_pool(name="lnqp", bufs=1) as lnqp:
                        if phases >= 1:
                            abq = _ln_stats(xqf, TQ, lnq_scratch, lnqp, "lnq")
                            for ci in range(CC):
                                tmp = lnqp.tile([P, TQ], BF16, tag="tmpq",
                                                name="lnq_t")
                                nc.vector.tensor_mul(tmp, xqf[:, ci, :],
                                                     abq[:, 0, :])
                                nc.vector.tensor_sub(tmp, tmp, abq[:, 1, :])
                                nc.vector.tensor_scalar(
                                    out=hq[:, ci, :], in0=tmp,
                                    scalar1=l1g_c[:, ci:ci + 1],
                                    scalar2=l1b_c[:, ci:ci + 1],
                                    op0=ALU.mult, op1=ALU.add)

                    # ------- Phase 1: LN1 over kv rows and query rows
                    with tc.tile_pool(name="ln1", bufs=1) as ln1p, \
                         tc.tile_pool(name="xkvB", bufs=1) as xkvB, \
                         tc.tile_pool(name="tmpB", bufs=2) as tmpB:
                        if phases >= 1:
                            xkv = xkvB.tile([P, CC, T], BF16, tag="xkv",
                                            name="xkv")
                            for ci in range(CC):
                                nc.sync.dma_start(
                                    out=xkv[:, ci, :],
                                    in_=xkvT[ci * P:(ci + 1) * P, :])
                            ab = _ln_stats(xkv, T, ln_scratch, ln1p, "ln1")
                            for ci in range(CC):
                                tmp = tmpB.tile([P, T], BF16, tag="tmp",
                                                name="ln1_t")
                                nc.vector.tensor_mul(tmp, xkv[:, ci, :],
                                                     ab[:, 0, :])
                                nc.vector.tensor_sub(tmp, tmp, ab[:, 1, :])
                                nc.vector.tensor_scalar(
                                    out=hT[:, ci, :], in0=tmp,
                                    scalar1=l1g_c[:, ci:ci + 1],
                                    scalar2=l1b_c[:, ci:ci + 1],
                                    op0=ALU.mult, op1=ALU.add)

                    with tc.tile_pool(name="wB", bufs=1) as wB, \
                         tc.tile_pool(name="qkB", bufs=1) as qkB, \
                         tc.tile_pool(name="yTB", bufs=1) as yTB:
                        if phases == 1:
                            o32 = qkB.tile([P, TQ], F32, tag="dbg1",
                                           name="dbg1")
                            nc.vector.tensor_scalar(out=o32,
                                                    in0=hT[:, 0, 0:TQ],
                                                    scalar1=one_c, scalar2=None,
                                                    op0=ALU.mult)
                            nc.sync.dma_start(out=outT[0:P, :], in_=o32)
                        # projection weights (fp8, DoubleRow-packed on host)
                        wq_t, wk_t, wv_t = [], [], []
                        for g in range(4 if phases >= 2 else 0):
                            for (lst, src, nm) in (
                                    (wq_t, Wq8, "wq"), (wk_t, Wk8, "wk"),
                                    (wv_t, Wv8, "wv")):
                                wt = wB.tile([P, 2, C], FP8, tag=f"{nm}{g}",
                                             name=f"{nm}{g}")
                                nc.sync.dma_start(
                                    out=wt,
                                    in_=src[g * P:(g + 1) * P, :].rearrange(
                                        "p (k m) -> p k m", k=2))
                                lst.append(wt)
                        wo_t = []
                        for ci in range(CC if phases >= 2 else 0):
                            wt = wB.tile([P, C], BF16, tag=f"wo{ci}",
                                         name=f"wo{ci}")
                            nc.sync.dma_start(
                                out=wt, in_=WoB[ci * P:(ci + 1) * P, :])
                            wo_t.append(wt)

                        # ------- Phase 2: Q/K/V projections (fp8 DR)
                        q8 = qkB.tile([P, CC, TQ], FP8, tag="q8", name="q8")
                        kp8 = [qkB.tile([P, T], FP8, tag=f"kp{pr}",
                                        name=f"kp{pr}") for pr in range(NPAIR)]
                        with tc.tile_pool(name="pps", bufs=4,
                                          space="PSUM") as pp:
                            if phases >= 2:
                                for co in range(CC):
                                    for tt in range(NQT):
                                        ps = pp.tile([P, 512], F32, tag="ps",
                                                     name="q_ps")
                                        for g in range(4):
                                            nc.tensor.matmul(
                                                ps,
                                                wq_t[g][:, :,
                                                        co * P:(co + 1) * P],
                                                hq[:, 2 * g:2 * g + 2,
                                                   tt * 512:(tt + 1) * 512],
                                                start=(g == 0), stop=(g == 3),
                                                perf_mode=DR)
                                        nc.vector.tensor_scalar(
                                            out=q8[:, co,
                                                   tt * 512:(tt + 1) * 512],
                                            in0=ps, scalar1=bq_c[:, co:co + 1],
                                            scalar2=None, op0=ALU.add)
                                for pr in range(NPAIR):
                                    for tt in range(4):
                                        ps = pp.tile([P, 512], F32, tag="ps",
                                                     name="k_ps")
                                        for g in range(4):
                                            nc.tensor.matmul(
                                                ps,
                                                wk_t[g][:, :,
                                                        pr * P:(pr + 1) * P],
                                                hT[:, 2 * g:2 * g + 2,
                                                   tt * 512:(tt + 1) * 512],
                                                start=(g == 0), stop=(g == 3),
                                                perf_mode=DR)
                                        nc.vector.tensor_scalar(
                                            out=kp8[pr][:,
                                                        tt * 512:(tt + 1) * 512],
                                            in0=ps, scalar1=bk_c[:, pr:pr + 1],
                                            scalar2=None, op0=ALU.add)
                                # V: stationary = hT chunk, moving = Wv
                                for tk in range(NKT):
                                    for ct in range(2):
                                        ps = pp.tile([P, 512], F32, tag="ps",
                                                     name="v_ps")
                                        for g in range(4):
                                            nc.tensor.matmul(
                                                ps,
                                                hT[:, 2 * g:2 * g + 2,
                                                   tk * P:(tk + 1) * P],
                                                wv_t[g][:, :,
                                                        ct * 512:(ct + 1) * 512],
                                                start=(g == 0), stop=(g == 3),
                                                perf_mode=DR)
                                        ps3 = ps.rearrange(
                                            "p (pr h d) -> p pr h d", h=2, d=D)
                                        bv3 = bv_b[:, ct * 512:(ct + 1) * 512
                                                   ].rearrange(
                                            "p (pr h d) -> p pr h d", h=2, d=D)
                                        for j in range(4):
                                            prr = ct * 4 + j
                                            dst = bass.AP(
                                                tensor=vg8[prr].tensor,
                                                offset=vg8[prr].offset
                                                + tk * 2 * VW,
                                                ap=[list(vg8[prr].ap[0]),
                                                    [VW, 2], [1, D]])
                                            nc.vector.tensor_add(
                                                dst, ps3[:, j], bv3[:, j])
                        if phases == 2:
                            o32 = qkB.tile([P, TQ], F32, tag="dbg2",
                                           name="dbg2")
                            nc.vector.tensor_scalar(out=o32, in0=q8[:, 0, :],
                                                    scalar1=one_c, scalar2=None,
                                                    op0=ALU.mult)
                            nc.sync.dma_start(out=outT[0:P, :], in_=o32)

                        # ------- Phase 3: attention
                        yT8 = yTB.tile([P, CC, TQ], BF16, tag="yT8", name="yT8")
                        with tc.tile_pool(name="sps", bufs=2,
                                          space="PSUM") as sps, \
                             tc.tile_pool(name="yps", bufs=3,
                                          space="PSUM") as ypsp, \
                             tc.tile_pool(name="rbp", bufs=1,
                                          space="PSUM") as rbp, \
                             tc.tile_pool(name="e8p", bufs=3) as e8p, \
                             tc.tile_pool(name="rcp", bufs=1) as rcp:
                            for pr in range(NPAIR if phases >= 3 else 0):
                                for hd in range(2):
                                    hsl = slice(hd * D, (hd + 1) * D)
                                    for tqt in range(NQT):
                                        qsl0 = tqt * 512
                                        nfull = 4 * tqt
                                        nbat = nfull + 4
                                        yps = ypsp.tile([VW, 512], F32,
                                                        tag="yps", name="yps")
                                        pend = None

                                        def _ymm(pe8, pjj, pym, stop):
                                            nc.tensor.matmul(
                                                yps[:, pym:512],
                                                vg8[pr][:,
                                                        2 * pjj:2 * pjj + 2,
                                                        hd * VW:(hd + 1) * VW],
                                                pe8[:, :, pym:512],
                                                start=(pjj == 0), stop=stop,
                                                perf_mode=DR)

                                        for jj in range(nbat):
                                            band = jj >= nfull
                                            w0 = 2 * (jj - nfull) if band else 0
                                            sm0 = D * w0 if band else 0
                                            sm1 = D * (w0 + 1) if band else 0
                                            sp = sps.tile([P, 1024], F32,
                                                          tag="sp",
                                                          name="att_sp")
                                            e8 = e8p.tile([P, 2, 512], FP8,
                                                          tag="e8",
                                                          name="att_e8")
                                            for half in range(2):
                                                smin = sm0 if half == 0 else sm1
                                                ck = 2 * jj + half
                                                nc.tensor.matmul(
                                                    sp[:, half * 512 + smin:
                                                       half * 512 + 512],
                                                    kp8[pr][hsl,
                                                            ck * P:(ck + 1) * P],
                                                    q8[hsl, pr,
                                                       qsl0 + smin:qsl0 + 512])
                                            if band:
                                                nc.vector.tensor_add(
                                                    sp[:, sm0:sm0 + D],
                                                    sp[:, sm0:sm0 + D], wedge_t)
                                                nc.vector.tensor_add(
                                                    sp[:, 512 + sm1:
                                                       512 + sm1 + D],
                                                    sp[:, 512 + sm1:
                                                       512 + sm1 + D], wedge_t)
                                                nc.scalar.activation(
                                                    e8[:, 0, sm0:],
                                                    sp[:, sm0:512],
                                                    AF.Exp, scale=EXP_SCALE)
                                                nc.scalar.activation(
                                                    e8[:, 1, sm1:],
                                                    sp[:, 512 + sm1:1024],
                                                    AF.Exp, scale=EXP_SCALE)
                                                if sm1 > sm0:
                                                    nc.vector.memset(
                                                        e8[:, 1, sm0:sm1], 0.0)
                                            else:
                                                nc.scalar.activation(
                                                    e8[:, :, :], sp[:, :],
                                                    AF.Exp, scale=EXP_SCALE)
                                            if pend is not None:
                                                _ymm(*pend, stop=False)
                                            pend = (e8, jj, sm0)
                                        _ymm(*pend, stop=True)
                                        # normalize: yT = y[0:64] / y[64]
                                        rrow = rcp.tile([D + 1, 512], BF16,
                                                        tag="rr", name="rrow")
                                        nc.vector.reciprocal(rrow[D:D + 1, :],
                                                             yps[D:D + 1, :])
                                        rb = rbp.tile([D, 512], F32, tag="rb",
                                                      name="rb")
                                        nc.tensor.matmul(rb,
                                                         ones_all[D:D + 1, :],
                                                         rrow[D:D + 1, :])
                                        # DVE reads at most one PSUM input:
                                        # stage rb in SBUF via ScalarE
                                        rbs = rcp.tile([D, 512], BF16,
                                                       tag="rbs", name="rbs")
                                        nc.scalar.activation(rbs, rb, AF.Copy)
                                        if hd == 0:
                                            nc.vector.tensor_mul(
                                                yT8[0:D, pr, qsl0:qsl0 + 512],
                                                yps[0:D, :], rbs)
                                        else:
                                            # DVE cannot shift partitions:
                                            # stage rows 0:64, DMA to 64:128
                                            yst = rcp.tile([D, 512], BF16,
                                                           tag="yst",
                                                           name="yst")
                                            nc.vector.tensor_mul(
                                                yst, yps[0:D, :], rbs)
                                            nc.sync.dma_start(
                                                out=yT8[D:P, pr,
                                                        qsl0:qsl0 + 512],
                                                in_=yst)
                        if phases == 3:
                            o32 = yTB.tile([P, TQ], F32, tag="dbg3",
                                           name="dbg3")
                            nc.vector.tensor_scalar(out=o32, in0=yT8[:, 0, :],
                                                    scalar1=one_c, scalar2=None,
                                                    op0=ALU.mult)
                            nc.sync.dma_start(out=outT[0:P, :], in_=o32)

                        # ------- Phase 4a: output proj + residual -> x2
                        x2 = x2B.tile([P, CC, TQ], F32, tag="x2", name="x2")
                        with tc.tile_pool(name="ops", bufs=4,
                                          space="PSUM") as pp, \
                             tc.tile_pool(name="otmp", bufs=2) as otp:
                            if phases >= 4:
                                for co in range(CC):
                                    for tt in range(NQT):
                                        sl = slice(tt * 512, (tt + 1) * 512)
                                        ps = pp.tile([P, 512], F32, tag="ps",
                                                     name="o_ps")
                                        for ci in range(CC):
                                            nc.tensor.matmul(
                                                ps,
                                                wo_t[ci][:,
                                                         co * P:(co + 1) * P],
                                                yT8[:, ci, sl],
                                                start=(ci == 0),
                                                stop=(ci == CC - 1))
                                        ot = otp.tile([P, 512], F32, tag="ot",
                                                      name="o_t")
                                        nc.vector.tensor_scalar(
                                            out=ot, in0=ps,
                                            scalar1=bo_c[:, co:co + 1],
                                            scalar2=None, op0=ALU.add)
                                        nc.vector.tensor_add(
                                            x2[:, co, sl].bitcast(F32R), ot,
                                            xqf[:, co, sl])
                        if phases == 4:
                            o32 = yTB.tile([P, TQ], F32, tag="dbg4",
                                           name="dbg4")
                            nc.vector.tensor_scalar(out=o32, in0=x2[:, 0, :],
                                                    scalar1=one_c, scalar2=None,
                                                    op0=ALU.mult)
                            nc.sync.dma_start(out=outT[0:P, :], in_=o32)

                # ------- Phase 4b/5: LN2 + MLP (hB/wB/qkB/yTB freed)
                with tc.tile_pool(name="h2B", bufs=1) as h2B, \
                     tc.tile_pool(name="ln2p", bufs=1) as ln2p, \
                     tc.tile_pool(name="tmp2", bufs=2) as tmp2B:
                    h2 = h2B.tile([P, CC, TQ], BF16, tag="h2", name="h2")
                    if phases >= 5:
                        ab2 = _ln_stats(x2, TQ, ln2_scratch, ln2p, "ln2")
                        for ci in range(CC):
                            tmp = tmp2B.tile([P, TQ], BF16, tag="tmp",
                                             name="ln2_t")
                            nc.vector.tensor_mul(tmp, x2[:, ci, :],
                                                 ab2[:, 0, :])
                            nc.vector.tensor_sub(tmp, tmp, ab2[:, 1, :])
                            nc.vector.tensor_scalar(
                                out=h2[:, ci, :], in0=tmp,
                                scalar1=l2g_c[:, ci:ci + 1],
                                scalar2=l2b_c[:, ci:ci + 1],
                                op0=ALU.mult, op1=ALU.add)

                    with tc.tile_pool(name="m1B", bufs=1) as m1B, \
                         tc.tile_pool(name="w12", bufs=1) as w12, \
                         tc.tile_pool(name="w1p", bufs=2) as w1p, \
                         tc.tile_pool(name="mps", bufs=3, space="PSUM") as mps, \
                         tc.tile_pool(name="oout", bufs=2) as oout:
                        m18 = m1B.tile([P, NHC, TQ], FP8, tag="m18",
                                       name="m18")
                        w2_t = []
                        for g in range(16 if phases >= 5 else 0):
                            wt = w12.tile([P, 2, C], FP8, tag=f"w2_{g}",
                                          name=f"w2_{g}")
                            nc.sync.dma_start(
                                out=wt,
                                in_=W28[g * P:(g + 1) * P, :].rearrange(
                                    "p (k m) -> p k m", k=2))
                            w2_t.append(wt)
                        for hhg in range(4 if phases >= 5 else 0):
                            w1g = []
                            for ci in range(CC):
                                wt = w1p.tile([P, 1024], BF16, tag=f"w1c{ci}",
                                              name=f"w1c{ci}")
                                nc.sync.dma_start(
                                    out=wt,
                                    in_=W1B[ci * P:(ci + 1) * P,
                                            hhg * 1024:(hhg + 1) * 1024])
                                w1g.append(wt)
                            for hl in range(8):
                                hh = hhg * 8 + hl
                                hsl2 = slice(hl * P, (hl + 1) * P)
                                ps = mps.tile([P, 1024], F32, tag="mp",
                                              name="m1_ps")
                                for tt in range(NQT):
                                    for ci in range(CC):
                                        nc.tensor.matmul(
                                            ps[:, tt * 512:(tt + 1) * 512],
                                            w1g[ci][:, hsl2],
                                            h2[:, ci,
                                               tt * 512:(tt + 1) * 512],
                                            start=(ci == 0), stop=(ci == 7))
                                nc.scalar.activation(
                                    m18[:, hh, :], ps, AF.Gelu,
                                    bias=b1_c[:, hh:hh + 1], scale=1.0)
                        for co in range(CC if phases >= 5 else 0):
                            ps = mps.tile([P, 1024], F32, tag="mp",
                                          name="m2_ps")
                            for tt in range(NQT):
                                for g in range(16):
                                    nc.tensor.matmul(
                                        ps[:, tt * 512:(tt + 1) * 512],
                                        w2_t[g][:, :, co * P:(co + 1) * P],
                                        m18[:, 2 * g:2 * g + 2,
                                            tt * 512:(tt + 1) * 512],
                                        start=(g == 0), stop=(g == 15),
                                        perf_mode=DR)
                            ot = oout.tile([P, TQ], F32, tag="ot",
                                           name="m_out")
                            nc.vector.tensor_scalar(
                                out=ot, in0=ps, scalar1=sinv_c,
                                scalar2=b2_c[:, co:co + 1],
                                op0=ALU.mult, op1=ALU.add)
                            nc.vector.tensor_add(ot, ot, x2[:, co, :])
                            nc.sync.dma_start(
                                out=outT[co * P:(co + 1) * P, :], in_=ot)

        for _it in range(iters):
            _emit_body()

    nc.compile()
    return nc


_NC_CACHE = None


def _get_nc():
    global _NC_CACHE
    if _NC_CACHE is None:
        _NC_CACHE = build_program()
    return _NC_CACHE


def _pack_w(W):
    """[Cin, Cout] fp32 -> DoubleRow-packed fp8 [Cin/2, 2*Cout], x32."""
    import ml_dtypes
    Cin, Cout = W.shape
    G = Cin // 256
    Wp = (WS * W).reshape(G, 2, 128, Cout).transpose(0, 2, 1, 3)
    return np.ascontiguousarray(Wp.reshape(G * 128, 2 * Cout)).astype(
        ml_dtypes.float8_e4m3)


def make_in_maps(x, ln1_g, ln1_b, Wq, bq, Wk, bk, Wv, bv, Wo, bo,
                 ln2_g, ln2_b, W1, b1, W2, b2):
    import ml_dtypes
    BF = ml_dtypes.bfloat16
    x = np.asarray(x, np.float32)
    shared = dict(
        Wq8=_pack_w(np.asarray(Wq, np.float32)),
        Wk8=_pack_w(np.asarray(Wk, np.float32)),
        Wv8=_pack_w(np.asarray(Wv, np.float32)),
        WoB=np.asarray(Wo, np.float32).astype(BF),
        W1B=np.asarray(W1, np.float32).astype(BF),
        W28=_pack_w(np.asarray(W2, np.float32)),
        bq32=WS * np.asarray(bq, np.float32),
        bk32=WS * np.asarray(bk, np.float32),
        bv32=WS * np.asarray(bv, np.float32),
        bo=np.asarray(bo, np.float32), b1=np.asarray(b1, np.float32),
        b2=np.asarray(b2, np.float32),
        ln1_g=np.asarray(ln1_g, np.float32),
        ln1_b=np.asarray(ln1_b, np.float32),
        ln2_g=np.asarray(ln2_g, np.float32),
        ln2_b=np.asarray(ln2_b, np.float32),
    )
    kk = np.arange(P)[:, None]
    ii = np.arange(D)[None, :]
    wedges = {h: np.where(kk <= 2 * ii + h, 0.0, MASK_VAL).astype(np.float32)
              for h in range(2)}
    in_maps = []
    for c in range(N_CORES):
        b, par = c // 2, c % 2
        xb = x[b]
        m = dict(shared)
        m["xkvT"] = np.ascontiguousarray(xb.T).astype(BF)
        m["xqF"] = np.ascontiguousarray(xb[par::2].T)
        m["wedge"] = wedges[par]
        in_maps.append(m)
    return in_maps


def kernel(x, ln1_g, ln1_b, Wq, bq, Wk, bk, Wv, bv, Wo, bo,
           ln2_g, ln2_b, W1, b1, W2, b2):
    nc = _get_nc()
    in_maps = make_in_maps(x, ln1_g, ln1_b, Wq, bq, Wk, bk, Wv, bv, Wo, bo,
                           ln2_g, ln2_b, W1, b1, W2, b2)
    res = run_bass_kernel_spmd(nc, in_maps, core_ids=list(range(N_CORES))).results
    B = 4
    out = np.empty((B, T, C), np.float32)
    for c in range(N_CORES):
        b, par = c // 2, c % 2
        out[b, par::2, :] = res[c]["outT"].T
    return out



# revision 17
# speedup vs baseline: 1.2957x; 1.2957x over previous
"""Trainium2 Bass kernel for a pre-LN transformer block (B=4, T=2048, C=1024,
H=16 heads, MLP 4x), data-parallel over 8 NeuronCores.

Sharding: core c <- (batch b = c//2, parity = c%2); each core handles query
rows x[b, parity::2] plus LayerNorm1 + K/V projections over the full T=2048
rows of its batch (duplicated across the parity pair; no collectives).

v3 (from v2 baseline at 905us/core):
- Host folds ln1_g/b into Wq/Wk/Wv + biases, ln2_g/b into W1 + b1; LN apply
  becomes 2 DVE ops (x*a - ma) direct to fp8.
- Pairwise column-swap trick: odd-parity cores get xkvT with adjacent token
  columns swapped so q tokens are ALWAYS the even columns -> the whole
  query-side LN (lnq) is deleted; Q projection reads hT with stride-2 moving
  APs. The per-parity wedge input already encodes the mask difference.
- Reciprocals: attention denominators via DVE reciprocal_approx_fast (was
  3.8us DVE RECIPROCAL each); LN rstd via ACT exp(-0.5*ln(var+eps)) -- ln/exp
  share one ACT table so no table thrash.
- Scores for the two heads of a pair issued back-to-back as K=64 matmuls at
  tile_position (0,0)/(64,0) -> concurrent in the PE array (2x score rate).
- V-bias epilogue batched: one vgAll tile, one [128,4,2,64] DVE add per
  (tk,ct) instead of 4 small ones; wedge adds merged across heads.
- LN stats postprocess on [1,T] rows trimmed to 4 ops; mean/E[x2] come
  directly out of PSUM via 1/C-valued stationary vectors.
"""

import numpy as np
from contextlib import ExitStack

import concourse.bass as bass
import concourse.bacc as bacc
import concourse.tile as tile
from concourse import mybir
from concourse.bass_utils import run_bass_kernel_spmd

F32 = mybir.dt.float32
F32R = mybir.dt.float32r
BF16 = mybir.dt.bfloat16
FP8 = mybir.dt.float8e4
AF = mybir.ActivationFunctionType
ALU = mybir.AluOpType
DR = mybir.MatmulPerfMode.DoubleRow

C = 1024          # embedding dim
T = 2048          # kv sequence length per core
TQ = 1024         # query rows per core
H = 16            # heads
D = 64            # head dim
HID = 4096        # mlp hidden
P = 128           # partitions
CC = C // P       # 8 feature chunks
NKT = T // P      # 16 key chunks
NQT = TQ // 512   # 2 query tiles of 512
NPAIR = H // 2    # 8 head pairs (= feature chunks)
NHC = HID // P    # 32 hidden chunks
EPS = 1e-5
WS = 32.0         # host-side fp8 weight prescale
SINV = 1.0 / WS
MASK_VAL = -8.0e9
EXP_SCALE = 0.125 / (WS * WS)   # q,k both carry x32

N_CORES = 8
VW = 80           # padded per-head vg width (64 v dims + aug@64 + pad); %16==0
VG_PR = NKT * 2 * VW   # vgAll stride per head-pair (2560)


def _bcast2(dram_ap, parts, rows, n):
    """[parts, rows, n] AP broadcasting a [rows, n] dram region across parts."""
    return bass.AP(tensor=dram_ap.tensor, offset=dram_ap.offset,
                   ap=[[0, parts], [n, rows], [1, n]])


def build_program(phases=None):
    import os
    if phases is None:
        phases = int(os.environ.get("BASS_BUILD_PHASES", "5"))
    nc = bacc.Bacc("TRN2", target_bir_lowering=False, debug=False)

    xkvT = nc.declare_dram_parameter("xkvT", [C, T], BF16, False)
    xqF = nc.declare_dram_parameter("xqF", [C, TQ], F32, False)
    wedge = nc.declare_dram_parameter("wedge", [P, D], F32, False)
    Wq8 = nc.declare_dram_parameter("Wq8", [512, 2 * C], FP8, False)
    Wk8 = nc.declare_dram_parameter("Wk8", [512, 2 * C], FP8, False)
    Wv8 = nc.declare_dram_parameter("Wv8", [512, 2 * C], FP8, False)
    WoB = nc.declare_dram_parameter("WoB", [C, C], BF16, False)
    W1B = nc.declare_dram_parameter("W1B", [C, HID], BF16, False)
    W28 = nc.declare_dram_parameter("W28", [2048, 2 * C], FP8, False)
    bq32 = nc.declare_dram_parameter("bq32", [C], F32, False)
    bk32 = nc.declare_dram_parameter("bk32", [C], F32, False)
    bv32 = nc.declare_dram_parameter("bv32", [C], F32, False)
    bo = nc.declare_dram_parameter("bo", [C], F32, False)
    b1 = nc.declare_dram_parameter("b1", [HID], F32, False)
    b2 = nc.declare_dram_parameter("b2", [C], F32, False)
    outT = nc.declare_dram_parameter("outT", [C, TQ], F32, True)
    taps = int(os.environ.get("BASS_DEBUG_TAPS", "0"))
    dbg = {}
    if taps:
        for nm, dt in (("dbg_hT", FP8), ("dbg_q8", FP8), ("dbg_kp0", FP8),
                       ("dbg_yT8", BF16), ("dbg_x2", F32)):
            dbg[nm] = nc.declare_dram_parameter(nm, [P, TQ], dt, True)

    ln_scratch = nc.dram_tensor("ln_scratch", [2, T], BF16)
    ln2_scratch = nc.dram_tensor("ln2_scratch", [2, TQ], BF16)

    with tile.TileContext(nc) as tc, ExitStack() as ctx:
        ctx.enter_context(nc.allow_low_precision(
            reason="fp8/bf16 kernel; tolerance 2e-2"))
        consts = ctx.enter_context(tc.tile_pool(name="consts", bufs=1))

        def load_cols(v, n, nm):
            t = consts.tile([P, n], F32, tag=nm, name=nm)
            nc.sync.dma_start(out=t, in_=v.rearrange("(k p) -> p k", p=P))
            return t

        bq_c = load_cols(bq32, CC, "bq_c")
        bk_c = load_cols(bk32, CC, "bk_c")
        bo_c = load_cols(bo, CC, "bo_c")
        b2_c = load_cols(b2, CC, "b2_c")
        b1_c = load_cols(b1, NHC, "b1_c")
        bv_b = consts.tile([P, C], F32, tag="bv_b", name="bv_b")
        nc.sync.dma_start(out=bv_b, in_=_bcast2(bv32[:], P, 1, C))
        wedge_t = consts.tile([P, D], F32, tag="wedge", name="wedge")
        nc.sync.dma_start(out=wedge_t, in_=wedge[:, :])
        # wedge2: two side-by-side copies for the merged 2-head wedge add
        wedge2 = consts.tile([P, 2, D], F32, tag="wedge2", name="wedge2")
        nc.sync.dma_start(out=wedge2[:, 0, :], in_=wedge[:, :])
        nc.sync.dma_start(out=wedge2[:, 1, :], in_=wedge[:, :])
        # 1/C-valued stationary vectors for LN stats (exact in bf16: 2^-10)
        onesC_bf = consts.tile([P, 1], BF16, tag="onesC_bf", name="onesC_bf")
        nc.scalar.activation(onesC_bf, bv_b[:, 0:1], AF.Copy,
                             bias=1.0 / C, scale=0.0)
        onesC_r = consts.tile([P, 1], F32, tag="onesC_r", name="onesC_r")
        nc.scalar.activation(onesC_r.bitcast(F32R), bv_b[:, 0:1], AF.Copy,
                             bias=1.0 / C, scale=0.0)
        ones_rb = consts.tile([P, D], BF16, tag="ones_rb", name="ones_rb")
        nc.scalar.activation(ones_rb, bv_b[:, 0:D], AF.Copy, bias=1.0,
                             scale=0.0)
        one_c = consts.tile([P, 1], F32, tag="one_c", name="one_c")
        nc.vector.memset(one_c, 1.0)
        eps_t = consts.tile([1, 1], F32, tag="eps", name="eps")
        nc.vector.memset(eps_t, EPS)
        sinv_c = consts.tile([P, 1], F32, tag="sinv_c", name="sinv_c")
        nc.vector.memset(sinv_c, SINV)



        def _ln_postproc(sum_ps, sq_ps, t_len, scratch, pool, nm):
            """[1,t_len] PSUM mean / E[x^2] -> bf16 (rstd, mean*rstd) rows in
            dram scratch. ln/exp only (no act-table swaps)."""
            msq = pool.tile([1, t_len], F32, tag=f"{nm}_msq", name=f"{nm}_msq")
            nc.scalar.activation(msq, sum_ps, AF.Square)
            var = pool.tile([1, t_len], F32, tag=f"{nm}_var", name=f"{nm}_var")
            nc.vector.tensor_sub(var, sq_ps, msq)
            lnv = pool.tile([1, t_len], F32, tag=f"{nm}_lnv", name=f"{nm}_lnv")
            nc.scalar.activation(lnv, var, AF.Ln, bias=eps_t)
            a_bf = pool.tile([1, t_len], BF16, tag=f"{nm}_a", name=f"{nm}_a")
            nc.scalar.activation(a_bf, lnv, AF.Exp, scale=-0.5)
            ma_bf = pool.tile([1, t_len], BF16, tag=f"{nm}_ma", name=f"{nm}_ma")
            nc.vector.tensor_mul(ma_bf, sum_ps, a_bf)
            nc.gpsimd.dma_start(out=scratch[0:1, :], in_=a_bf)
            nc.gpsimd.dma_start(out=scratch[1:2, :], in_=ma_bf)
            ab = pool.tile([P, 2, t_len], BF16, tag=f"{nm}_ab", name=f"{nm}_ab")
            nc.sync.dma_start(out=ab, in_=_bcast2(scratch[:, :], P, 2, t_len))
            return ab

        def _tap(nm, ap, pool):
            if not taps:
                return
            nc.sync.dma_start(out=dbg[nm][:, :], in_=ap)

        with tc.tile_pool(name="x2B", bufs=1) as x2B:
            with tc.tile_pool(name="hB", bufs=1) as hB:
                hT = hB.tile([P, CC, T], FP8, tag="hT", name="hT")
                xqf = hB.tile([P, CC, TQ], F32, tag="xqf", name="xqf")
                for ci in range(CC):
                    nc.sync.dma_start(
                        out=xqf[:, ci, :].bitcast(F32R),
                        in_=xqF[ci * P:(ci + 1) * P, :].bitcast(F32R))

                # ------- Phase 1: LN1 (stats + standardize) over kv rows
                with tc.tile_pool(name="ln1", bufs=1) as ln1p, \
                     tc.tile_pool(name="xkvB", bufs=1) as xkvB, \
                     tc.tile_pool(name="sqB", bufs=2) as sqB, \
                     tc.tile_pool(name="tmpB", bufs=2) as tmpB:
                    if phases >= 1:
                        xkv = xkvB.tile([P, CC, T], BF16, tag="xkv",
                                        name="xkv")
                        for ci in range(CC):
                            nc.sync.dma_start(
                                out=xkv[:, ci, :],
                                in_=xkvT[ci * P:(ci + 1) * P, :])
                        with tc.tile_pool(name="ln1ps", bufs=1,
                                          space="PSUM") as psp:
                            sum_ps = psp.tile([1, T], F32, tag="ln1_su",
                                              name="ln1_su")
                            sq_ps = psp.tile([1, T], F32, tag="ln1_sq",
                                             name="ln1_sq")
                            for ci in range(CC):
                                sq = sqB.tile([P, T], BF16, tag="sq",
                                              name="ln1_sqt")
                                nc.scalar.activation(sq, xkv[:, ci, :],
                                                     AF.Square)
                                for t in range(T // 512):
                                    sl = slice(t * 512, (t + 1) * 512)
                                    nc.tensor.matmul(
                                        sum_ps[:, sl], onesC_bf,
                                        xkv[:, ci, sl],
                                        start=(ci == 0), stop=(ci == CC - 1))
                                    nc.tensor.matmul(
                                        sq_ps[:, sl], onesC_bf, sq[:, sl],
                                        start=(ci == 0), stop=(ci == CC - 1))
                            ab = _ln_postproc(sum_ps, sq_ps, T, ln_scratch,
                                              ln1p, "ln1")
                        for ci in range(CC):
                            eng = nc.vector
                            tmp = tmpB.tile([P, T], BF16, tag="tmp",
                                            name="ln1_t")
                            eng.tensor_mul(tmp, xkv[:, ci, :], ab[:, 0, :])
                            eng.tensor_sub(hT[:, ci, :], tmp, ab[:, 1, :])

                with tc.tile_pool(name="wB", bufs=1) as wB, \
                     tc.tile_pool(name="qkB", bufs=1) as qkB, \
                     tc.tile_pool(name="yTB", bufs=1) as yTB:
                    _tap("dbg_hT", hT[:, 0, 0:TQ], qkB)
                    if phases == 1:
                        o32 = qkB.tile([P, TQ], F32, tag="dbg1", name="dbg1")
                        nc.vector.tensor_scalar(out=o32, in0=hT[:, 0, 0:TQ],
                                                scalar1=one_c, scalar2=None,
                                                op0=ALU.mult)
                        nc.sync.dma_start(out=outT[0:P, :], in_=o32)
                    # projection weights (fp8, DoubleRow-packed on host)
                    wq_t, wk_t, wv_t = [], [], []
                    for g in range(4 if phases >= 2 else 0):
                        for (lst, src, nm) in (
                                (wq_t, Wq8, "wq"), (wk_t, Wk8, "wk"),
                                (wv_t, Wv8, "wv")):
                            wt = wB.tile([P, 2, C], FP8, tag=f"{nm}{g}",
                                         name=f"{nm}{g}")
                            nc.sync.dma_start(
                                out=wt,
                                in_=src[g * P:(g + 1) * P, :].rearrange(
                                    "p (k m) -> p k m", k=2))
                            lst.append(wt)
                    wo_t = []
                    for ci in range(CC if phases >= 2 else 0):
                        wt = wB.tile([P, C], BF16, tag=f"wo{ci}",
                                     name=f"wo{ci}")
                        nc.sync.dma_start(
                            out=wt, in_=WoB[ci * P:(ci + 1) * P, :])
                        wo_t.append(wt)

                    # ------- Phase 2: Q/K/V projections (fp8 DR)
                    vgAll = qkB.tile([P, NPAIR, NKT, 2, VW], FP8,
                                     tag="vgAll", name="vgAll")
                    # aug col (64) = WS; pad cols (65:80) also WS (their PSUM
                    # rows 65:79 are never read) -- one strided memset
                    nc.vector.memset(
                        bass.AP(tensor=vgAll.tensor, offset=vgAll.offset + D,
                                ap=[list(vgAll.ap[0]),
                                    [VW, NPAIR * NKT * 2], [1, VW - D]]), WS)
                    q8 = qkB.tile([P, CC, TQ], FP8, tag="q8", name="q8")
                    kp8 = [qkB.tile([P, T], FP8, tag=f"kp{pr}",
                                    name=f"kp{pr}") for pr in range(NPAIR)]
                    with tc.tile_pool(name="pps", bufs=4,
                                      space="PSUM") as pp:
                        if phases >= 2:
                            for pr in range(NPAIR):
                                for tt in range(4):
                                    ps = pp.tile([P, 512], F32, tag="ps",
                                                 name="k_ps")
                                    for g in range(4):
                                        nc.tensor.matmul(
                                            ps,
                                            wk_t[g][:, :,
                                                    pr * P:(pr + 1) * P],
                                            hT[:, 2 * g:2 * g + 2,
                                               tt * 512:(tt + 1) * 512],
                                            start=(g == 0), stop=(g == 3),
                                            perf_mode=DR)
                                    nc.vector.tensor_scalar(
                                        out=kp8[pr][:,
                                                    tt * 512:(tt + 1) * 512],
                                        in0=ps, scalar1=bk_c[:, pr:pr + 1],
                                        scalar2=None, op0=ALU.add)
                            # V: stationary = hT chunk, moving = Wv
                            for tk in range(NKT):
                                for ct in range(2):
                                    ps = pp.tile([P, 512], F32, tag="ps",
                                                 name="v_ps")
                                    for g in range(4):
                                        nc.tensor.matmul(
                                            ps,
                                            hT[:, 2 * g:2 * g + 2,
                                               tk * P:(tk + 1) * P],
                                            wv_t[g][:, :,
                                                    ct * 512:(ct + 1) * 512],
                                            start=(g == 0), stop=(g == 3),
                                            perf_mode=DR)
                                    ps3 = bass.AP(
                                        tensor=ps.tensor, offset=ps.offset,
                                        ap=[list(ps.ap[0]), [P, 4], [D, 2],
                                            [1, D]])
                                    bv3 = bass.AP(
                                        tensor=bv_b.tensor,
                                        offset=bv_b.offset + ct * 512,
                                        ap=[list(bv_b.ap[0]), [P, 4], [D, 2],
                                            [1, D]])
                                    dst = bass.AP(
                                        tensor=vgAll.tensor,
                                        offset=vgAll.offset + ct * 4 * VG_PR
                                        + tk * 2 * VW,
                                        ap=[list(vgAll.ap[0]), [VG_PR, 4],
                                            [VW, 2], [1, D]])
                                    nc.vector.tensor_add(dst, ps3, bv3)
                            # Q: moving reads hT strided (q tokens = even
                            # columns thanks to the host pairwise swap)
                            for co in range(CC):
                                for tt in range(NQT):
                                    ps = pp.tile([P, 512], F32, tag="ps",
                                                 name="q_ps")
                                    for g in range(4):
                                        mov = bass.AP(
                                            tensor=hT.tensor,
                                            offset=hT.offset + 2 * g * T
                                            + tt * 1024,
                                            ap=[list(hT.ap[0]), [T, 2],
                                                [2, 512]])
                                        nc.tensor.matmul(
                                            ps,
                                            wq_t[g][:, :,
                                                    co * P:(co + 1) * P],
                                            mov,
                                            start=(g == 0), stop=(g == 3),
                                            perf_mode=DR)
                                    nc.vector.tensor_scalar(
                                        out=q8[:, co,
                                               tt * 512:(tt + 1) * 512],
                                        in0=ps, scalar1=bq_c[:, co:co + 1],
                                        scalar2=None, op0=ALU.add)
                    _tap("dbg_q8", q8[:, 0, :], qkB)
                    _tap("dbg_kp0", kp8[0][:, 0:TQ], qkB)
                    if phases == 2:
                        o32 = qkB.tile([P, TQ], F32, tag="dbg2", name="dbg2")
                        nc.vector.tensor_scalar(out=o32, in0=q8[:, 0, :],
                                                scalar1=one_c, scalar2=None,
                                                op0=ALU.mult)
                        nc.sync.dma_start(out=outT[0:P, :], in_=o32)

                    # ------- Phase 3: attention (2-head-concurrent scores)
                    yT8 = yTB.tile([P, CC, TQ], BF16, tag="yT8", name="yT8")
                    with tc.tile_pool(name="sps", bufs=2,
                                      space="PSUM") as sps, \
                         tc.tile_pool(name="yps", bufs=3,
                                      space="PSUM") as ypsp, \
                         tc.tile_pool(name="rbp", bufs=1,
                                      space="PSUM") as rbp, \
                         tc.tile_pool(name="e8p", bufs=3) as e8p, \
                         tc.tile_pool(name="rcp", bufs=2) as rcp:
                        for pr in range(NPAIR if phases >= 3 else 0):
                            for tqt in range(NQT):
                                qsl0 = tqt * 512
                                nck = 8 * tqt + 8   # 128-key chunks
                                yps = [ypsp.tile([VW, 512], F32, tag="yps",
                                                 name="yps")
                                       for _ in range(2)]
                                pend = None

                                def _ymm(pe8, pm, pym, stop):
                                    for hd in range(2):
                                        mov = bass.AP(
                                            tensor=pe8.tensor,
                                            offset=pe8.offset + hd * 1024
                                            + pym,
                                            ap=[list(pe8.ap[0]), [512, 2],
                                                [1, 512 - pym]])
                                        nc.tensor.matmul(
                                            yps[hd][:, pym:512],
                                            bass.AP(
                                                tensor=vgAll.tensor,
                                                offset=vgAll.offset
                                                + pr * VG_PR + 2 * pm * 2 * VW
                                                + hd * VW,
                                                ap=[list(vgAll.ap[0]),
                                                    [2 * VW, 2], [1, VW]]),
                                            mov,
                                            start=(pm == 0), stop=stop,
                                            perf_mode=DR)

                                for m in range(nck // 2):
                                    sp_a = sps.tile([P, 1024], F32, tag="sp",
                                                    name="att_sp")
                                    sp_b = sps.tile([P, 1024], F32,
                                                    tag="sp", name="att_spb")
                                    e8 = e8p.tile([P, 2, 2, 512], FP8,
                                                  tag="e8", name="att_e8")
                                    sps_pair = (sp_a, sp_b)
                                    smins = []
                                    for half in range(2):
                                        ck = 2 * m + half
                                        w = ck - (nck - 8)
                                        band = w >= 0
                                        smin = D * w if (band and w > 0) else 0
                                        smins.append(smin)
                                        spx = sps_pair[half]
                                        for hd in range(2):
                                            hsl = slice(hd * D, hd * D + D)
                                            nc.tensor.matmul(
                                                spx[:, hd * 512 + smin:
                                                    hd * 512 + 512],
                                                kp8[pr][hsl,
                                                        ck * P:(ck + 1) * P],
                                                q8[hsl, pr,
                                                   qsl0 + smin:qsl0 + 512])
                                        if band:
                                            # merged 2-head wedge add
                                            dstw = bass.AP(
                                                tensor=spx.tensor,
                                                offset=spx.offset + smin,
                                                ap=[list(spx.ap[0]),
                                                    [512, 2], [1, D]])
                                            nc.vector.tensor_add(
                                                dstw, dstw, wedge2)
                                        # exp both heads in one ACT op
                                        nc.scalar.activation(
                                            bass.AP(
                                                tensor=e8.tensor,
                                                offset=e8.offset
                                                + half * 512 + smin,
                                                ap=[list(e8.ap[0]),
                                                    [1024, 2],
                                                    [1, 512 - smin]]),
                                            bass.AP(
                                                tensor=spx.tensor,
                                                offset=spx.offset + smin,
                                                ap=[list(spx.ap[0]),
                                                    [512, 2],
                                                    [1, 512 - smin]]),
                                            AF.Exp, scale=EXP_SCALE)
                                    if smins[1] > smins[0]:
                                        # zero odd-chunk cols the AV matmul
                                        # reads below its smin
                                        nc.vector.memset(
                                            bass.AP(
                                                tensor=e8.tensor,
                                                offset=e8.offset + 512
                                                + smins[0],
                                                ap=[list(e8.ap[0]),
                                                    [1024, 2], [1, D]]), 0.0)
                                    if pend is not None:
                                        _ymm(*pend, stop=False)
                                    pend = (e8, m, smins[0])
                                _ymm(*pend, stop=True)

                                # normalize: yT = y[0:64] / y[64]
                                for hd in range(2):
                                    rtmp = rcp.tile([P, 512], F32, tag="rt",
                                                    name="rtmp")
                                    rt16 = rcp.tile([P, 512], BF16,
                                                    tag="rt16", name="rt16")
                                    # 1/d = exp(-ln(d)); ln+exp share an ACT
                                    # table with the attention exps
                                    nc.scalar.activation(
                                        rtmp[D:D + 1, :],
                                        yps[hd][D:D + 1, :], AF.Ln)
                                    nc.scalar.activation(
                                        rt16[D:D + 1, :],
                                        rtmp[D:D + 1, :], AF.Exp, scale=-1.0)
                                    rb = rbp.tile([D, 512], F32, tag="rb",
                                                  name="rb")
                                    nc.tensor.matmul(
                                        rb,
                                        ones_rb[D:D + 1, :],
                                        rt16[D:D + 1, :])
                                    rbs = rcp.tile([D, 512], BF16, tag="rbs",
                                                   name="rbs")
                                    nc.scalar.activation(rbs, rb, AF.Copy)
                                    if hd == 0:
                                        nc.vector.tensor_mul(
                                            yT8[0:D, pr, qsl0:qsl0 + 512],
                                            yps[hd][0:D, :], rbs)
                                    else:
                                        # DVE cannot shift partitions:
                                        # stage rows 0:64, DMA to 64:128
                                        yst = rcp.tile([D, 512], BF16,
                                                       tag="yst", name="yst")
                                        nc.vector.tensor_mul(
                                            yst, yps[hd][0:D, :], rbs)
                                        nc.sync.dma_start(
                                            out=yT8[D:P, pr,
                                                    qsl0:qsl0 + 512],
                                            in_=yst)
                    _tap("dbg_yT8", yT8[:, 0, :], yTB)
                    if phases == 3:
                        o32 = yTB.tile([P, TQ], F32, tag="dbg3", name="dbg3")
                        nc.vector.tensor_scalar(out=o32, in0=yT8[:, 0, :],
                                                scalar1=one_c, scalar2=None,
                                                op0=ALU.mult)
                        nc.sync.dma_start(out=outT[0:P, :], in_=o32)

                    # ------- Phase 4a: output proj + residual -> x2
                    x2 = x2B.tile([P, CC, TQ], F32, tag="x2", name="x2")
                    with tc.tile_pool(name="ops", bufs=4,
                                      space="PSUM") as pp2:
                        if phases >= 4:
                            for co in range(CC):
                                for tt in range(NQT):
                                    sl = slice(tt * 512, (tt + 1) * 512)
                                    ps = pp2.tile([P, 512], F32, tag="ps",
                                                  name="o_ps")
                                    for ci in range(CC):
                                        nc.tensor.matmul(
                                            ps,
                                            wo_t[ci][:,
                                                     co * P:(co + 1) * P],
                                            yT8[:, ci, sl],
                                            start=(ci == 0),
                                            stop=(ci == CC - 1))
                                    nc.vector.scalar_tensor_tensor(
                                        out=x2[:, co, sl].bitcast(F32R),
                                        in0=ps,
                                        scalar=bo_c[:, co:co + 1],
                                        in1=xqf[:, co, sl],
                                        op0=ALU.add, op1=ALU.add)
                    _tap("dbg_x2", x2[:, 0, :], yTB)
                    if phases == 4:
                        o32 = yTB.tile([P, TQ], F32, tag="dbg4", name="dbg4")
                        nc.vector.tensor_scalar(out=o32, in0=x2[:, 0, :],
                                                scalar1=one_c, scalar2=None,
                                                op0=ALU.mult)
                        nc.sync.dma_start(out=outT[0:P, :], in_=o32)

            # ------- Phase 4b/5: LN2 + MLP (hB/wB/qkB/yTB freed)
            with tc.tile_pool(name="h2B", bufs=1) as h2B, \
                 tc.tile_pool(name="ln2p", bufs=1) as ln2p, \
                 tc.tile_pool(name="sq2B", bufs=2) as sq2B, \
                 tc.tile_pool(name="tmp2", bufs=2) as tmp2B:
                h2 = h2B.tile([P, CC, TQ], BF16, tag="h2", name="h2")
                if phases >= 5:
                    with tc.tile_pool(name="ln2ps", bufs=1,
                                      space="PSUM") as psp2:
                        sum2 = psp2.tile([1, TQ], F32, tag="ln2_su",
                                         name="ln2_su")
                        sqp2 = psp2.tile([1, TQ], F32, tag="ln2_sq",
                                         name="ln2_sq")
                        for ci in range(CC):
                            sq = sq2B.tile([P, TQ], F32, tag="sq",
                                           name="ln2_sqt")
                            nc.scalar.activation(sq.bitcast(F32R),
                                                 x2[:, ci, :], AF.Square)
                            for t in range(TQ // 512):
                                sl = slice(t * 512, (t + 1) * 512)
                                nc.tensor.matmul(
                                    sum2[:, sl], onesC_r.bitcast(F32R),
                                    x2[:, ci, sl].bitcast(F32R),
                                    start=(ci == 0), stop=(ci == CC - 1))
                                nc.tensor.matmul(
                                    sqp2[:, sl], onesC_r.bitcast(F32R),
                                    sq.bitcast(F32R)[:, sl],
                                    start=(ci == 0), stop=(ci == CC - 1))
                        ab2 = _ln_postproc(sum2, sqp2, TQ, ln2_scratch,
                                           ln2p, "ln2")
                    for ci in range(CC):
                        eng = nc.vector
                        tmp = tmp2B.tile([P, TQ], BF16, tag="tmp",
                                         name="ln2_t")
                        eng.tensor_mul(tmp, x2[:, ci, :], ab2[:, 0, :])
                        eng.tensor_sub(h2[:, ci, :], tmp, ab2[:, 1, :])

                with tc.tile_pool(name="m1B", bufs=1) as m1B, \
                     tc.tile_pool(name="w12", bufs=1) as w12, \
                     tc.tile_pool(name="w1p", bufs=2) as w1p, \
                     tc.tile_pool(name="mps", bufs=3, space="PSUM") as mps, \
                     tc.tile_pool(name="oout", bufs=2) as oout:
                    m18 = m1B.tile([P, NHC, TQ], FP8, tag="m18", name="m18")
                    w2_t = []
                    for g in range(16 if phases >= 5 else 0):
                        wt = w12.tile([P, 2, C], FP8, tag=f"w2_{g}",
                                      name=f"w2_{g}")
                        nc.sync.dma_start(
                            out=wt,
                            in_=W28[g * P:(g + 1) * P, :].rearrange(
                                "p (k m) -> p k m", k=2))
                        w2_t.append(wt)
                    for hhg in range(4 if phases >= 5 else 0):
                        w1g = []
                        for ci in range(CC):
                            wt = w1p.tile([P, 1024], BF16, tag=f"w1c{ci}",
                                          name=f"w1c{ci}")
                            nc.sync.dma_start(
                                out=wt,
                                in_=W1B[ci * P:(ci + 1) * P,
                                        hhg * 1024:(hhg + 1) * 1024])
                            w1g.append(wt)
                        for hl in range(8):
                            hh = hhg * 8 + hl
                            hsl2 = slice(hl * P, (hl + 1) * P)
                            ps = mps.tile([P, 1024], F32, tag="mp",
                                          name="m1_ps")
                            for tt in range(NQT):
                                for ci in range(CC):
                                    nc.tensor.matmul(
                                        ps[:, tt * 512:(tt + 1) * 512],
                                        w1g[ci][:, hsl2],
                                        h2[:, ci,
                                           tt * 512:(tt + 1) * 512],
                                        start=(ci == 0), stop=(ci == 7))
                            nc.scalar.activation(
                                m18[:, hh, :], ps, AF.Gelu,
                                bias=b1_c[:, hh:hh + 1], scale=1.0)
                    for co in range(CC if phases >= 5 else 0):
                        ps = mps.tile([P, 1024], F32, tag="mp",
                                      name="m2_ps")
                        for tt in range(NQT):
                            for g in range(16):
                                nc.tensor.matmul(
                                    ps[:, tt * 512:(tt + 1) * 512],
                                    w2_t[g][:, :, co * P:(co + 1) * P],
                                    m18[:, 2 * g:2 * g + 2,
                                        tt * 512:(tt + 1) * 512],
                                    start=(g == 0), stop=(g == 15),
                                    perf_mode=DR)
                        ot = oout.tile([P, TQ], F32, tag="ot", name="m_out")
                        nc.vector.tensor_scalar(
                            out=ot, in0=ps, scalar1=sinv_c,
                            scalar2=b2_c[:, co:co + 1],
                            op0=ALU.mult, op1=ALU.add)
                        nc.vector.tensor_add(ot, ot, x2[:, co, :])
                        nc.sync.dma_start(
                            out=outT[co * P:(co + 1) * P, :], in_=ot)

    nc.compile()
    return nc


_NC_CACHE = None


def _get_nc():
    global _NC_CACHE
    if _NC_CACHE is None:
        _NC_CACHE = build_program()
    return _NC_CACHE


def _pack_w(W):
    """[Cin, Cout] fp32 -> DoubleRow-packed fp8 [Cin/2, 2*Cout], x32."""
    import ml_dtypes
    Cin, Cout = W.shape
    G = Cin // 256
    Wp = (WS * W).reshape(G, 2, 128, Cout).transpose(0, 2, 1, 3)
    return np.ascontiguousarray(Wp.reshape(G * 128, 2 * Cout)).astype(
        ml_dtypes.float8_e4m3)


def make_in_maps(x, ln1_g, ln1_b, Wq, bq, Wk, bk, Wv, bv, Wo, bo,
                 ln2_g, ln2_b, W1, b1, W2, b2):
    import ml_dtypes
    BF = ml_dtypes.bfloat16
    x = np.asarray(x, np.float32)
    g1 = np.asarray(ln1_g, np.float32)
    b1n = np.asarray(ln1_b, np.float32)
    g2 = np.asarray(ln2_g, np.float32)
    b2n = np.asarray(ln2_b, np.float32)
    Wq = np.asarray(Wq, np.float32)
    Wk = np.asarray(Wk, np.float32)
    Wv = np.asarray(Wv, np.float32)
    W1 = np.asarray(W1, np.float32)
    # fold LN affine into the projection weights / biases
    Wqf, Wkf, Wvf = g1[:, None] * Wq, g1[:, None] * Wk, g1[:, None] * Wv
    bqf = b1n @ Wq + np.asarray(bq, np.float32)
    bkf = b1n @ Wk + np.asarray(bk, np.float32)
    bvf = b1n @ Wv + np.asarray(bv, np.float32)
    W1f = g2[:, None] * W1
    b1f = b2n @ W1 + np.asarray(b1, np.float32)
    shared = dict(
        Wq8=_pack_w(Wqf), Wk8=_pack_w(Wkf), Wv8=_pack_w(Wvf),
        WoB=np.asarray(Wo, np.float32).astype(BF),
        W1B=W1f.astype(BF),
        W28=_pack_w(np.asarray(W2, np.float32)),
        bq32=WS * bqf, bk32=WS * bkf, bv32=WS * bvf,
        bo=np.asarray(bo, np.float32), b1=b1f,
        b2=np.asarray(b2, np.float32),
    )
    kk = np.arange(P)[:, None]
    ii = np.arange(D)[None, :]
    wedges = {h: np.where(kk <= 2 * ii + h, 0.0, MASK_VAL).astype(np.float32)
              for h in range(2)}
    in_maps = []
    for c in range(N_CORES):
        b, par = c // 2, c % 2
        xb = x[b]
        m = dict(shared)
        xkv = xb
        if par:
            # pairwise column swap: q tokens become the even columns
            xkv = xb.reshape(T // 2, 2, C)[:, ::-1, :].reshape(T, C)
        m["xkvT"] = np.ascontiguousarray(xkv.T).astype(BF)
        m["xqF"] = np.ascontiguousarray(xb[par::2].T)
        m["wedge"] = wedges[par]
        in_maps.append(m)
    return in_maps


def kernel(x, ln1_g, ln1_b, Wq, bq, Wk, bk, Wv, bv, Wo, bo,
           ln2_g, ln2_b, W1, b1, W2, b2):
    nc = _get_nc()
    in_maps = make_in_maps(x, ln1_g, ln1_b, Wq, bq, Wk, bk, Wv, bv, Wo, bo,
                           ln2_g, ln2_b, W1, b1, W2, b2)
    res = run_bass_kernel_spmd(nc, in_maps, core_ids=list(range(N_CORES))).results
    B = 4
    out = np.empty((B, T, C), np.float32)
    for c in range(N_CORES):
        b, par = c // 2, c % 2
        out[b, par::2, :] = res[c]["outT"].T
    return out
